# revision 19
# baseline (speedup 1.0000x reference)
"""CARAFE pair-quad kernel: 8 cores, one batch each.

Math per batch (KS=5, GS=1, SF=2):
  out[c, 4g+2dh+a, ow] = sum_{rel,dyi,w'} f[c, 2(g+rel-1)+dyi, w'] *
                         T[g, p=2w'+dyi, (rel, ow, dh, a)]
where T holds mask values m[5dy+dx, oh, ow] at dy = 2rel+dyi-dh,
dx = w'-ow//2+2 (zero outside valid ranges).

Device layout (octet-strip diet):
  fA   [128, FA_AL] bf16: p = 2w'+dyi, f = m*C + c  ->  f[2m+dyi, w', c]
  T    [128, T_AL] bf16 x4 ring (pre-zeroed once): col = op*768 + hp*192 +
       rel*64 + oin*32 + owl3*4 + (2dh+a), ow = 16*op + 8*oin + owl3.
       Only the Toeplitz band is shipped: per (block, octet-pair op) a strip
       of partitions [16op-4, 16op+20) x cols [768op, 768op+768); everything
       else stays zero (the strip rectangles are identical for every block,
       so zeroing happens once per ring slot at startup).
  psum [128, 512] f32 x6 ring: col j = ow*4 + 2dh + a
  outS [128, 4096] bf16 x3 ring: f = half*2048 + hp*512 + dh*256 + a*128 + ow
"""
import sys

if "/opt/trn_rl_repo" not in sys.path:
    sys.path.append("/opt/trn_rl_repo")

from contextlib import ExitStack

import numpy as np
import ml_dtypes

import concourse.bass as bass
import concourse.bacc as bacc
import concourse.mybir as mybir
import concourse.tile as tile
from concourse.ap import AP
from concourse.bass_utils import run_bass_kernel_spmd

N = 8
C = 256
H = 64
W = 64
NBLK = 8
HPB = 4
T_AL = 6144
FA_AL = 8192 + 256
FA0_M = 6                 # head tile pair-cols (unblocks first matmuls early)
OS_AL = 4096
TRING = 5
OSRING = 4
MERGED = 6 * 24 * 768                         # op=1..6 strips, 24 partitions each
BLK_STREAM = MERGED + 2 * 20 * 768            # + clipped op=0 / op=7 strips
TOPE_SZ = NBLK * BLK_STREAM


def _rap(tile_ap, off, dims):
    return AP(tile_ap.tensor, tile_ap.offset + off, dims)


def build_carafe(nc, repeat=1):
    feat = nc.declare_dram_parameter("features", (H, W, C), mybir.dt.bfloat16, isOutput=False)
    tope = nc.declare_dram_parameter("masks", (TOPE_SZ,), mybir.dt.bfloat16, isOutput=False)
    out = nc.declare_dram_parameter("out", (C, 2 * H, 2 * W), mybir.dt.bfloat16, isOutput=True)

    ctx = ExitStack()
    with ctx:
        tc = ctx.enter_context(tile.TileContext(nc))
        pool = ctx.enter_context(tc.tile_pool(name="main", bufs=1))
        ppool = ctx.enter_context(tc.tile_pool(name="psum", bufs=1, space="PSUM"))

        M1 = 32 - FA0_M
        fA0 = pool.tile([128, FA0_M * C + 64], mybir.dt.bfloat16, tag="fA0", name="fA0")
        fA1 = pool.tile([128, M1 * C + 64], mybir.dt.bfloat16, tag="fA1", name="fA1")

        def load_fa2(tile_, m0, mcnt, mcol0):
            pit = tile_.shape[1]
            for dyi in range(2):
                nc.scalar.dma_start(
                    _rap(tile_[:, :], dyi * pit + mcol0 * C,
                         [[2 * pit, 64], [C, mcnt], [1, C]]),
                    _rap(feat[:, :, :], (2 * m0 + dyi) * W * C,
                         [[C, 64], [2 * W * C, mcnt], [1, C]]))

        def load_fa(tile_, m0, mcnt):
            load_fa2(tile_, m0, mcnt, 0 if tile_ is not None and m0 == 0 else m0 - FA0_M)

        T = [pool.tile([128, T_AL], mybir.dt.bfloat16, tag=f"T{i}", name=f"T{i}")
             for i in range(TRING)]

        def zero_tile(t_):
            # cost ~ col extent; split halves across DVE and Pool
            hh = T_AL // 2
            nc.vector.memset(_rap(t_[:, :], 0, [[T_AL, 128], [1, hh]]), 0.0)
            nc.gpsimd.memset(_rap(t_[:, :], hh, [[T_AL, 128], [1, hh]]), 0.0)

        def load_strips(t_, b, engs):
            # DMA APs cannot step partitions diagonally, so one DMA per
            # octet-pair: strip op: partitions [16op-4, 16op+20),
            # cols [768op, 768op+768)
            base = b * BLK_STREAM
            for oi, op in enumerate(range(1, 7)):
                engs[oi % len(engs)].dma_start(
                    _rap(t_[:, :], (16 * op - 4) * T_AL + 768 * op,
                         [[T_AL, 24], [1, 768]]),
                    _rap(tope[:], base + oi * 24 * 768, [[768, 24], [1, 768]]))
            # clipped op=0 / op=7 strips
            engs[-1].dma_start(
                _rap(t_[:, :], 0, [[T_AL, 20], [1, 768]]),
                _rap(tope[:], base + MERGED, [[768, 20], [1, 768]]))
            engs[-1].dma_start(
                _rap(t_[:, :], 108 * T_AL + 7 * 768, [[T_AL, 20], [1, 768]]),
                _rap(tope[:], base + MERGED + 20 * 768, [[768, 20], [1, 768]]))

        # T[0] zero on DVE+ACT (fast, overlaps fA0 DMA) so block-0 strips can
        # land; split 2:1 to balance DVE 4x vs the slower ACT
        sp = 4096
        nc.vector.memset(_rap(T[0][:, :], 0, [[T_AL, 128], [1, sp]]), 0.0)
        nc.scalar.memzero(_rap(T[0][:, :], sp, [[T_AL, 128], [1, T_AL - sp]]))

        # PE warm-up during startup DMA wait: garbage matmuls on a scratch
        # tile (never written, no deps) keep the clock-gate warm so the
        # first real matmuls run at full rate.  N=64 keeps them short: they
        # must not delay block 0's real matmuls (the strip load they hide is
        # only ~0.3MB now).
        scratch = pool.tile([128, 640], mybir.dt.bfloat16, tag="warm", name="warm")
        pwarm = ppool.tile([128, 512], mybir.dt.float32, tag="pwarm", name="pwarm")
        nc.gpsimd.memset(scratch[:, :], 0.0)
        for _ in range(10):
            nc.tensor.matmul(pwarm[:, 0:64],
                             _rap(scratch[:, :], 0, [[640, 128], [1, 128]]),
                             _rap(scratch[:, :], 128, [[640, 128], [1, 64]]),
                             start=True, stop=True)
        load_fa(fA0, 0, FA0_M)
        zero_tile(T[1])
        outS = [pool.tile([128, OS_AL], mybir.dt.bfloat16, tag=f"oS{i}", name=f"oS{i}")
                for i in range(OSRING)]
        psum = [ppool.tile([128, 512], mybir.dt.float32, tag=f"ps{i}", name=f"ps{i}")
                for i in range(6)]

        for it in range(NBLK * repeat):
            b = it % NBLK
            t = T[it % TRING]
            oS = outS[it % OSRING]
            if it < 2:
                load_strips(t, b, [nc.sync])          # keep Pool free at startup
            else:
                load_strips(t, b, [nc.sync, nc.scalar, nc.gpsimd])
            # fA1 in quarters over blocks 0-3: block b only needs row-pairs
            # m <= 4b+4 (2-block lead on every piece), so the 1.7MB load
            # doesn't oversubscribe DMA during the first two blocks.
            if it == 0:
                load_fa2(fA1, 6, 7, 0)
                zero_tile(T[2])
            elif it == 1:
                load_fa2(fA1, 13, 7, 7)
                zero_tile(T[3])
            elif it == 2:
                load_fa2(fA1, 20, 6, 14)
                zero_tile(T[4])
            elif it == 3:
                load_fa2(fA1, 26, 6, 20)
            for hp in range(HPB):
                g = HPB * b + hp
                for half in (0, 1):
                    ps = psum[(2 * g + half) % 6]
                    rels = [r for r in range(3) if 0 <= g + r - 1 <= 31]
                    nrel = len(rels)
                    for i, rel in enumerate(rels):
                        m = g + rel - 1
                        if m < FA0_M:
                            fa, pit = fA0, FA0_M * C + 64
                            moff = m
                        else:
                            fa, pit = fA1, M1 * C + 64
                            moff = m - FA0_M
                        lhsT = _rap(fa[:, :], moff * C + half * 128, [[pit, 128], [1, 128]])
                        # free walk (op, ox): 128B-contiguous inner runs;
                        # psum col j = ow*4 + dha
                        rhs = _rap(t[:, :], hp * 192 + rel * 64,
                                   [[T_AL, 128], [768, 8], [1, 64]])
                        nc.tensor.matmul(ps[:, 0:512], lhsT, rhs,
                                         start=(i == 0), stop=(i == nrel - 1))
                    cp = nc.vector.tensor_copy if (g + half) % 2 == 0 else nc.scalar.copy
                    # oS col = dh*256 + a*128 + ow  <-  psum col = ow*4 + 2dh + a
                    cp(_rap(oS[:, :], half * 2048 + hp * 512, [[OS_AL, 128], [1, 512]]),
                       _rap(ps[:, :], 0, [[512, 128], [2, 2], [1, 2], [4, 128]]))
            # one output DMA per half (NOT per hp): HWDGE queues are FIFO, so
            # several copy-gated output DMAs head-of-line-block the next
            # block's strip prefetches on the same queue (measured +8us/rep)
            if it == NBLK * repeat - 1:
                # tail: one piece per copy so each DMA fires immediately
                for hp in range(HPB):
                    for half in (0, 1):
                        oeng = (nc.sync, nc.scalar, nc.gpsimd)[(2 * hp + half) % 3]
                        oeng.dma_start(
                            _rap(out[:, :, :],
                                 half * 128 * 16384 + (16 * b + 4 * hp) * 128,
                                 [[16384, 128], [1, 512]]),
                            _rap(oS[:, :], half * 2048 + hp * 512,
                                 [[OS_AL, 128], [1, 512]]))
            else:
                for half in (0, 1):
                    oeng = nc.sync if half == 0 else nc.scalar
                    oeng.dma_start(
                        _rap(out[:, :, :], half * 128 * 16384 + 16 * b * 128,
                             [[16384, 128], [1, 2048]]),
                        _rap(oS[:, :], half * 2048, [[OS_AL, 128], [1, 2048]]))
    return nc


def prep_features(features_f32):
    ft = np.ascontiguousarray(features_f32.transpose(0, 2, 3, 1)).astype(ml_dtypes.bfloat16)
    return [ft[i] for i in range(ft.shape[0])]


def _build_dense(masks):
    """masks (25,128,128) f32 -> D[g=32, p=128, rel=3, ow=128, dh=2, a=2]"""
    D = np.zeros((32, 128, 3, 128, 2, 2), np.float32)
    ow = np.arange(128)
    g = np.arange(32)
    for rel in range(3):
        r0 = 2 * g + 2 * (rel - 1)
        gv = (r0 >= 0) & (r0 <= 62)
        for dyi in range(2):
            for dh in range(2):
                dy = 2 * rel + dyi - dh
                if not (0 <= dy <= 4):
                    continue
                for dx in range(5):
                    wp = ow // 2 + dx - 2
                    v = (wp >= 0) & (wp < 64)
                    k = 5 * dy + dx
                    for a in range(2):
                        oh = 4 * g[:, None] + 2 * dh + a
                        vals = masks[k][oh, ow[None, :]] * gv[:, None]
                        D[:, 2 * wp[v] + dyi, rel, ow[v], dh, a] = vals[:, v]
    return D


def prep_masks(masks_f32):
    """(N, 25, 128, 128) f32 -> per-batch flat octet-strip stream bf16.

    Per block b: merged strips op=1..6 as (op, p 24, col 768), then the
    clipped op=0 strip (p 0..20) and op=7 strip (p 108..128), 768 cols each.
    Within a pair, col = hp*192 + rel*64 + oin*32 + owl3*4 + (2dh+a)."""
    n = masks_f32.shape[0]
    outs = []
    for i in range(n):
        D = _build_dense(masks_f32[i])              # (g, p, rel, ow, dh, a)
        chunks = []
        for b in range(NBLK):
            Tb = D[4 * b:4 * b + 4]                  # (hp, p, rel, ow, dh, a)
            arr = Tb.reshape(4, 128, 3, 8, 2, 8, 2, 2)  # hp, p, rel, op, oin, owl3, dh, a
            arr = arr.transpose(1, 3, 0, 2, 4, 5, 6, 7)  # p, op, hp, rel, oin, owl3, dh, a
            Tfull = np.ascontiguousarray(arr).reshape(128, 8, 768)
            merged = np.stack([Tfull[16 * op - 4:16 * op + 20, op] for op in range(1, 7)])
            chunks.append(merged.reshape(-1))
            chunks.append(Tfull[0:20, 0].reshape(-1))
            chunks.append(Tfull[108:128, 7].reshape(-1))
        outs.append(np.concatenate(chunks).astype(ml_dtypes.bfloat16))
    return outs


_NC_CACHE = {}


def _get_nc(repeat=1):
    key = ("nc", repeat)
    if key not in _NC_CACHE:
        nc = bacc.Bacc()
        build_carafe(nc, repeat=repeat)
        nc.compile()
        _NC_CACHE[key] = nc
    return _NC_CACHE[key]


def _in_maps(features, masks):
    fts = prep_features(np.asarray(features, dtype=np.float32))
    mbs = prep_masks(np.asarray(masks, dtype=np.float32))
    return [{"features": fts[i], "masks": mbs[i]} for i in range(N)]


def run_profiled(inputs):
    """Run with NTFF tracing; returns exec_time_ns (or None if unavailable)."""
    nc = _get_nc()
    res = run_bass_kernel_spmd(nc, _in_maps(inputs["features"], inputs["masks"]),
                               core_ids=list(range(N)), trace=True)
    return res.exec_time_ns


def bench(features, masks, reps=64, repeat=1):
    """Repeat-execute the compiled NEFF on all 8 cores; returns (per_iter_ns,
    first_call_s).  Upper bound on HW exec time (includes dispatch overhead)."""
    import time
    import jax
    from jax.sharding import Mesh, PartitionSpec
    from jax.experimental.shard_map import shard_map
    from concourse import bass2jax
    import concourse.mybir as mybir_

    nc = _get_nc(repeat)
    bass2jax.install_neuronx_cc_hook()
    in_maps = _in_maps(features, masks)

    in_names, out_names, out_avals, zero_outs = [], [], [], []
    for alloc in nc.m.functions[0].allocations:
        if not isinstance(mybir_.MemoryLocationSet, type) or not isinstance(alloc, mybir_.MemoryLocationSet):
            continue
        name = alloc.memorylocations[0].name
        pname = nc.partition_id_tensor.name if nc.partition_id_tensor else None
        if alloc.kind == "ExternalInput":
            if name != pname:
                in_names.append(name)
        elif alloc.kind == "ExternalOutput":
            out_names.append(name)
            shape = tuple(alloc.tensor_shape)
            dtype = mybir_.dt.np(alloc.dtype)
            out_avals.append(jax.core.ShapedArray(shape, dtype))
            zero_outs.append(np.zeros(shape, dtype))
    n_params = len(in_names)
    in_names = in_names + out_names
    if nc.partition_id_tensor is not None:
        in_names.append(nc.partition_id_tensor.name)

    def _body(*args):
        operands = list(args)
        if nc.partition_id_tensor is not None:
            operands.append(bass2jax.partition_id_tensor())
        outs = bass2jax._bass_exec_p.bind(
            *operands,
            out_avals=tuple(out_avals),
            in_names=tuple(in_names),
            out_names=tuple(out_names),
            lowering_input_output_aliases=(),
            sim_require_finite=True,
            sim_require_nnan=True,
            nc=nc,
        )
        return tuple(outs)

    devices = jax.devices()[:N]
    mesh = Mesh(np.asarray(devices), ("core",))
    nin = n_params + len(out_names)
    fn = jax.jit(
        shard_map(_body, mesh=mesh, in_specs=(PartitionSpec("core"),) * nin,
                  out_specs=(PartitionSpec("core"),) * len(out_names),
                  check_rep=False),
        keep_unused=True,
    )
    per_core = [[np.asarray(m[k]) for k in in_names[:n_params]] for m in in_maps]
    args = [np.concatenate([per_core[c][i] for c in range(N)], axis=0)
            for i in range(n_params)]
    args += [np.zeros((N * z.shape[0], *z.shape[1:]), z.dtype) for z in zero_outs]
    from jax.sharding import NamedSharding
    sh = NamedSharding(mesh, PartitionSpec("core"))
    args = [jax.device_put(a, sh) for a in args]
    t0 = time.time()
    outs = fn(*args)
    jax.block_until_ready(outs)
    first_s = time.time() - t0
    t0 = time.time()
    last = None
    for _ in range(reps):
        last = fn(*args)
    jax.block_until_ready(last)
    per_iter_ns = (time.time() - t0) / reps * 1e9
    return per_iter_ns, first_s



def kernel(features: np.ndarray, masks: np.ndarray) -> np.ndarray:
    nc = _get_nc()
    res = run_bass_kernel_spmd(nc, _in_maps(features, masks), core_ids=list(range(N)))
    return np.stack([np.asarray(res.results[i]["out"], dtype=np.float32)
                     for i in range(N)])
